# revision 1
# baseline (speedup 1.0000x reference)
"""GateRetention Trainium2 kernel (Bass/Tile), 8-core tensor-parallel.

Sharding: core grid (batch b = core//4, head-group g = core%4); each core owns
4 heads (512 cols of the q/k/v/g projections, 512 rows of Wo) of one batch.
RMS-norm statistics are AllReduced across each batch's 4 cores; out-proj
partials are summed on the host (row-parallel TP gather).

Precision: projections in fp32r (tf32-like); q/k/v/g staged to DRAM in fp16;
retention + out-proj in fp16 with fp32 PSUM accumulation; a 2^±10 exponent
shift on vfac/rowfac keeps the decayed v tiles inside fp16 range.

kernel(**inputs) takes the FULL inputs from reference.setup_inputs() and
returns the FULL [B, T, DIM] fp32 output.
"""
import os
import sys

sys.path.insert(0, "/opt/trn_rl_repo")

import numpy as np

import concourse.bass as bass
import concourse.bacc as bacc
import concourse.tile as tile
import concourse.mybir as mybir
from concourse import bass_utils

F32 = mybir.dt.float32
F32R = mybir.dt.float32r
F16 = mybir.dt.float16
AX = mybir.AxisListType
ALU = mybir.AluOpType
ACTF = mybir.ActivationFunctionType

B, T, DIM = 2, 4096, 2048
H, HD = 16, 128
CS = 256
NCH = T // CS              # 16 chunks
EPS = 1e-5
GLN = 16.0
SCALE = HD ** -0.5
NCORE = 8
HPC = 4                    # heads per core
PCOLS = HPC * HD           # 512 cols per core
NBLK = T // 128            # 32 token blocks of 128
VSH = 2.0 ** -2            # fp16 range shift on vv; inverse folded into rowfac

DEBUG_LVL = int(os.environ.get("GR_DEBUG", "0"))
DEBUG = bool(DEBUG_LVL)
DMASPLIT = int(os.environ.get("GR_DMASPLIT", "3"))
TRACE = bool(int(os.environ.get("GR_TRACE", "0")))

_cache = {}


def _consts_np():
    """[128, 520] fp32: identity | Lm | Om | Umask | ones."""
    ident = np.eye(128, dtype=np.float32)
    jj, ii = np.meshgrid(np.arange(128), np.arange(128), indexing="ij")
    Lm = np.where(jj <= ii, -1.0 / GLN, 0.0).astype(np.float32)
    Om = np.full((128, 128), -1.0 / GLN, np.float32)
    Um = np.where(jj <= ii, 1.0, 0.0).astype(np.float32)
    ones = np.ones((128, 8), np.float32)
    # Lc: b_i - b_mid for block0 = +1/GLN * sum_{j>i} sp_j
    Lc = np.where(jj > ii, 1.0 / GLN, 0.0).astype(np.float32)
    return np.concatenate([ident, Lm, Om, Um, ones, Lc], axis=1)


def build(debug=False):
    nc = bacc.Bacc("TRN2", target_bir_lowering=False, debug=False,
                   enable_asserts=False, num_devices=NCORE)

    # ---------------- I/O ----------------
    xT = nc.dram_tensor("xT", [DIM, T], F32R, kind="ExternalInput").ap()
    cT = nc.dram_tensor("cT", [DIM, T], F32R, kind="ExternalInput").ap()
    wq = nc.dram_tensor("wq", [DIM, PCOLS], F32R, kind="ExternalInput").ap()
    wk = nc.dram_tensor("wk", [DIM, PCOLS], F32R, kind="ExternalInput").ap()
    wv = nc.dram_tensor("wv", [DIM, PCOLS], F32R, kind="ExternalInput").ap()
    wg = nc.dram_tensor("wg", [DIM, PCOLS], F32R, kind="ExternalInput").ap()
    wgt = nc.dram_tensor("wgt", [DIM, HPC], F32R, kind="ExternalInput").ap()
    wo = nc.dram_tensor("wo", [PCOLS, DIM], F16, kind="ExternalInput").ap()
    consts = nc.dram_tensor("consts", [128, 648], F32R, kind="ExternalInput").ap()
    c16 = nc.dram_tensor("c16", [128, 128], F16, kind="ExternalInput").ap()
    out = nc.dram_tensor("out", [T, DIM], F16, kind="ExternalOutput").ap()

    def dbg(name, shape, dtype=F32):
        return nc.dram_tensor(name, shape, dtype, kind="ExternalOutput").ap()

    def _dma_gpsimd(*a, **k):
        eng = nc.gpsimd if DMASPLIT in (1, 2) else nc.sync
        return eng.dma_start(*a, **k)

    def _dma_scalar(*a, **k):
        eng = nc.scalar if DMASPLIT in (1, 3) else nc.sync
        return eng.dma_start(*a, **k)

    with tile.TileContext(nc) as tc:
        with (
            tc.tile_pool(name="const", bufs=1) as cpool,
            tc.tile_pool(name="wts", bufs=1) as wpool,
            tc.tile_pool(name="xstream", bufs=2) as xpool,
            tc.tile_pool(name="cstream", bufs=2) as ctpool,
            tc.tile_pool(name="evac", bufs=2) as epool,
            tc.tile_pool(name="persist", bufs=1) as ppool,
            tc.tile_pool(name="small", bufs=2) as spool,
            tc.tile_pool(name="ret", bufs=2) as rpool,
            tc.tile_pool(name="ps", bufs=1, space="PSUM") as psp,
            tc.tile_pool(name="dram", bufs=1, space="DRAM") as dpool,
        ):
            def ps_big():
                return psp.tile([128, 512], F32, tag="big", bufs=4, name="psbig")

            def ps_small(shape=None, dtype=F32):
                return psp.tile(shape or [128, 256], dtype, tag="small", bufs=4,
                                name="pssmall")

            # ---------------- constants ----------------
            cst = cpool.tile([128, 648], F32R, tag="consts")
            nc.sync.dma_start(cst[:], consts)
            ident = cst[:, 0:128]
            ident32 = ident.bitcast(F32)
            Lm = cst[:, 128:256]
            Om = cst[:, 256:384]
            Um = cst[:, 384:512]
            Um32 = Um.bitcast(F32)
            ones1 = cst[:, 512:513]
            Lc = cst[:, 520:648]
            i16 = cpool.tile([128, 128], F16, tag="i16")
            nc.sync.dma_start(i16[:], c16)

            # ---------------- DRAM scratch (fp16) ----------------
            if debug:
                qT_s = dbg("dbg_qT", [PCOLS, T], F16)
                kT_s = dbg("dbg_kT", [PCOLS, T], F16)
                vN_s = dbg("dbg_vN", [T, PCOLS], F16)
                gT_s = dbg("dbg_gT", [PCOLS, T], F16)
            else:
                qT_s = nc.dram_tensor("qT_s", [PCOLS, T], F16,
                                      kind="Internal").ap()
                kT_s = nc.dram_tensor("kT_s", [PCOLS, T], F16,
                                      kind="Internal").ap()
                vN_s = nc.dram_tensor("vN_s", [T, PCOLS], F16,
                                      kind="Internal").ap()
                gT_s = nc.dram_tensor("gT_s", [PCOLS, T], F16,
                                      kind="Internal").ap()
            ss_in = nc.dram_tensor("ss_in", [3, T], F32, kind="Internal").ap()
            ss_out = nc.dram_tensor("ss_out", [3, T], F32, kind="Internal").ap()

            # =========================================================
            # P1: projections (two passes), fp32r math, fp16 staging
            # =========================================================
            NT = T // 512  # 8 token n-tiles

            gtn = ppool.tile([128, NBLK, HPC], F32, tag="gtn")
            vss = ppool.tile([128, NBLK], F32, tag="vss")

            def load_w(wdram, tag):
                wt = wpool.tile([128, 16, 512], F32R, tag=tag)
                nc.sync.dma_start(
                    wt[:], wdram.rearrange("(kt p) m -> p kt m", p=128))
                return wt

            def xt_halves(n):
                tok = slice(n * 512, (n + 1) * 512)
                halves = []
                for h2 in range(2):
                    xt = xpool.tile([128, 8, 512], F32R, tag="xt")
                    nc.sync.dma_start(
                        xt[:], xT[h2 * 1024:(h2 + 1) * 1024, tok].rearrange(
                            "(kt p) m -> p kt m", p=128))
                    halves.append(xt)
                return halves

            def tproj_mms(ps, wt, xth, m):
                for k in range(16):
                    nc.tensor.matmul(
                        ps[:], wt[:, k, m * 128:(m + 1) * 128],
                        xth[k // 8][:, k % 8, :], start=(k == 0), stop=(k == 15))

            # ---------- pass A: q, k (T-layout) ----------
            wts_a = [load_w(wq, "w0"), load_w(wk, "w1")]
            for n in range(NT):
                tok = slice(n * 512, (n + 1) * 512)
                xth = xt_halves(n)
                for pi, sdram in enumerate((qT_s, kT_s)):
                    for m in range(4):
                        ps = ps_big()
                        tproj_mms(ps, wts_a[pi], xth, m)
                        ev = epool.tile([128, 512], F16, tag="ev")
                        sqt = epool.tile([128, 512], F32R, tag="sq")
                        if m % 2 == 0:
                            nc.vector.tensor_copy(ev[:], ps[:])
                            _dma_gpsimd(
                                sdram[m * 128:(m + 1) * 128, tok], ev[:])
                        else:
                            nc.scalar.copy(ev[:], ps[:])
                            _dma_scalar(
                                sdram[m * 128:(m + 1) * 128, tok], ev[:])
                        nc.scalar.activation(sqt[:], ps[:], ACTF.Square)
                        if m == 0:
                            ssps = ps_small([1, 512])
                        nc.tensor.matmul(ssps[:1, :], ones1, sqt[:],
                                         start=(m == 0), stop=(m == 3))
                        if m == 3:
                            ssev = spool.tile([1, 512], F32, tag="ssev", bufs=2)
                            nc.vector.tensor_copy(ssev[:], ssps[:1, :])
                            _dma_gpsimd(ss_in[pi:pi + 1, tok], ssev[:])

            # ---------- pass B: v natural, silu(g) T-layout, gt ----------
            wv_sb = load_w(wv, "w0")
            wg_sb = load_w(wg, "w1")
            wgt_sb = wpool.tile([128, 16, HPC], F32R, tag="wgt")
            nc.sync.dma_start(wgt_sb[:],
                              wgt.rearrange("(kt p) m -> p kt m", p=128))
            for n in range(NT):
                tok = slice(n * 512, (n + 1) * 512)
                xth = xt_halves(n)
                # v natural
                for mt in range(4):
                    msl = slice(mt * 128, (mt + 1) * 128)
                    ps = ps_big()
                    for k in range(16):
                        nc.tensor.matmul(
                            ps[:], xth[k // 8][:, k % 8, msl], wv_sb[:, k, :],
                            start=(k == 0), stop=(k == 15))
                    ev = epool.tile([128, 512], F16, tag="ev")
                    sqt = epool.tile([128, 512], F32R, tag="sq")
                    nc.scalar.activation(
                        sqt[:], ps[:], ACTF.Square,
                        accum_out=vss[:, n * 4 + mt:n * 4 + mt + 1])
                    if mt % 2 == 0:
                        nc.vector.tensor_copy(ev[:], ps[:])
                        _dma_gpsimd(
                            vN_s[n * 512 + mt * 128:n * 512 + (mt + 1) * 128, :],
                            ev[:])
                    else:
                        nc.scalar.copy(ev[:], ps[:])
                        _dma_scalar(
                            vN_s[n * 512 + mt * 128:n * 512 + (mt + 1) * 128, :],
                            ev[:])
                # silu(g), T-layout
                for m in range(4):
                    ps = ps_big()
                    tproj_mms(ps, wg_sb, xth, m)
                    ev = epool.tile([128, 512], F16, tag="ev")
                    nc.scalar.activation(ev[:], ps[:], ACTF.Silu)
                    _dma_scalar(gT_s[m * 128:(m + 1) * 128, tok], ev[:])
                # gt logits: accumulate x and c streams
                gtps = ps_small([128, 512])
                for k in range(16):
                    nc.tensor.matmul(gtps[:HPC, :], wgt_sb[:, k, :],
                                     xth[k // 8][:, k % 8, :],
                                     start=(k == 0), stop=False)
                for k in range(16):
                    ct = ctpool.tile([128, 512], F32R, tag="ct", bufs=8)
                    nc.sync.dma_start(ct[:], cT[k * 128:(k + 1) * 128, tok])
                    nc.tensor.matmul(gtps[:HPC, :], wgt_sb[:, k, :], ct[:],
                                     start=False, stop=(k == 15))
                gstg = spool.tile([HPC, 512], F32, tag="gstg", bufs=2)
                nc.vector.tensor_copy(gstg[:], gtps[:HPC, :])
                for j in range(4):
                    tp = ps_small([128, HPC])
                    nc.tensor.matmul(tp[:], gstg[:, j * 128:(j + 1) * 128],
                                     ident32[:HPC, :HPC], is_transpose=True)
                    nc.vector.tensor_copy(gtn[:, n * 4 + j, :], tp[:])

            # v sumsq: transpose [128, 32] -> [32, 128] -> ss_in row 2
            vssT = ps_small([128, 128])
            nc.tensor.matmul(vssT[:32, :], vss[:], ident32, is_transpose=True)
            vssev = spool.tile([32, 128], F32, tag="vssev", bufs=1)
            nc.vector.tensor_copy(vssev[:], vssT[:32, :])
            _dma_gpsimd(
                ss_in[2:3, :].rearrange("a (b c) -> (a b) c", c=128), vssev[:])

            # =========================================================
            # P2: AllReduce sumsq; scales; gate decays
            # =========================================================
            nc.gpsimd.collective_compute(
                "AllReduce", ALU.add,
                replica_groups=[[0, 1, 2, 3], [4, 5, 6, 7]],
                ins=[ss_in.opt()], outs=[ss_out.opt()],
            )
            ssn = ppool.tile([128, NBLK, 3], F32, tag="ssn")
            for nn_ in range(NT):
                tok = slice(nn_ * 512, (nn_ + 1) * 512)
                srt = spool.tile([3, 512], F32, tag="srt", bufs=1)
                nc.sync.dma_start(srt[:], ss_out[:, tok])
                for j in range(4):
                    tp = ps_small([128, 4])
                    nc.tensor.matmul(tp[:, :3], srt[:, j * 128:(j + 1) * 128],
                                     ident32[:3, :3], is_transpose=True)
                    nc.vector.tensor_copy(ssn[:, nn_ * 4 + j, :], tp[:, :3])
            rsn = ppool.tile([128, NBLK, 3], F32, tag="rsn")
            nc.vector.tensor_scalar(rsn[:], ssn[:], 1.0 / DIM, EPS,
                                    ALU.mult, ALU.add)
            nc.scalar.activation(rsn[:], rsn[:], ACTF.Ln)
            nc.scalar.activation(rsn[:], rsn[:], ACTF.Exp, scale=-0.5)
            skv = ppool.tile([128, NBLK], F32, tag="skv")
            nc.vector.tensor_mul(skv[:], rsn[:, :, 1], rsn[:, :, 2])
            if debug and DEBUG_LVL >= 2:
                nc.sync.dma_start(dbg("dbg_rsn", [128, NBLK * 3]),
                                  rsn[:].rearrange("p a b -> p (a b)"))

            # gate decays: sp = softplus(-z) = ln(1 + exp(-z)); -1/GLN in Lm/Om
            gtd = ppool.tile([128, NBLK, HPC], F32R, tag="gtd")
            nc.scalar.activation(gtn[:], gtn[:], ACTF.Exp, scale=-1.0)
            nc.scalar.activation(gtd[:], gtn[:], ACTF.Ln, bias=1.0)

            # per chunk: recentered b' = b - b_mid via triangular matmuls;
            # eS = exp(mid-to-mid decay) for the state recurrence
            rf = ppool.tile([128, NCH, 2, HPC], F32, tag="rf")      # rowfac
            vf = ppool.tile([128, NCH, 2, HPC], F32, tag="vf")      # vfac
            eS = ppool.tile([128, NCH, HPC], F32, tag="eS")
            for ch in range(NCH):
                b0, b1 = 2 * ch, 2 * ch + 1
                p0 = ps_small([128, HPC])
                nc.tensor.matmul(p0[:], Lc, gtd[:, b0, :], start=True, stop=True)
                p1 = ps_small([128, HPC])
                nc.tensor.matmul(p1[:], Lm, gtd[:, b1, :], start=True, stop=True)
                if ch < NCH - 1:
                    pt = ps_small([128, HPC])
                    nc.tensor.matmul(pt[:], Om, gtd[:, b1, :],
                                     start=True, stop=False)
                    nc.tensor.matmul(pt[:], Om, gtd[:, b1 + 1, :],
                                     start=False, stop=True)
                    nc.scalar.activation(eS[:, ch, :], pt[:], ACTF.Exp)
                for blk01, bps in ((0, p0), (1, p1)):
                    blk = 2 * ch + blk01
                    # rowfac = exp(b') * sq * scale / VSH
                    nc.scalar.activation(rf[:, ch, blk01, :], bps[:], ACTF.Exp)
                    nc.vector.tensor_scalar(
                        rf[:, ch, blk01, :], rf[:, ch, blk01, :],
                        rsn[:, blk, 0:1], SCALE / VSH, ALU.mult, ALU.mult)
                    # vfac = exp(-b') * sk * sv * VSH
                    nc.scalar.activation(vf[:, ch, blk01, :], bps[:], ACTF.Exp,
                                         scale=-1.0)
                    nc.vector.tensor_scalar(
                        vf[:, ch, blk01, :], vf[:, ch, blk01, :],
                        skv[:, blk:blk + 1], VSH, ALU.mult, ALU.mult)

            if debug and DEBUG_LVL >= 3:
                nc.sync.dma_start(
                    dbg("dbg_rf", [128, NCH * 2 * HPC]),
                    rf[:].rearrange("p a b c -> p (a b c)"))
                nc.sync.dma_start(
                    dbg("dbg_vf", [128, NCH * 2 * HPC]),
                    vf[:].rearrange("p a b c -> p (a b c)"))
                nc.sync.dma_start(
                    dbg("dbg_eS", [128, NCH * HPC]),
                    eS[:].rearrange("p a b -> p (a b)"))
            if debug and DEBUG_LVL >= 2:
                nc.sync.dma_start(
                    dbg("dbg_gtd", [128, NBLK * HPC]),
                    gtd[:].bitcast(F32).rearrange("p a b -> p (a b)"))

            # =========================================================
            # P3: retention + gating + out-proj, per chunk (fp16)
            # =========================================================
            if int(os.environ.get("GR_BARRIER", "0")):
                tc.prologue_barrier()
            wo_sb = wpool.tile([128, HPC, DIM], F16, tag="wo")
            nc.sync.dma_start(wo_sb[:], wo.rearrange("(h p) m -> p h m", p=128))

            S_prev = [None] * HPC
            for ch in range(NCH):
                tok = slice(ch * CS, (ch + 1) * CS)
                qc = rpool.tile([128, HPC, CS], F16, tag="qc")
                kc = rpool.tile([128, HPC, CS], F16, tag="kc")
                for t_, s_ in ((qc, qT_s), (kc, kT_s)):
                    nc.sync.dma_start(
                        t_[:], s_[:, tok].rearrange("(h p) m -> p h m", p=128))
                vcn, sg = [], []
                for blk01 in range(2):
                    bt = slice(ch * CS + blk01 * 128, ch * CS + blk01 * 128 + 128)
                    vt = rpool.tile([128, PCOLS], F16, tag="vcn")
                    nc.sync.dma_start(vt[:], vN_s[bt, :])
                    vcn.append(vt)
                    gt_ = rpool.tile([128, HPC, 128], F16, tag="gch")
                    nc.sync.dma_start(
                        gt_[:], gT_s[:, bt].rearrange("(h p) m -> p h m", p=128))
                    sg.append(gt_)
                o_st = rpool.tile([128, 2 * HPC, HD], F32, tag="o_st")
                for hl in range(HPC):
                    # k_nat via PE transpose; vv from natural v
                    knat, vvt = [], []
                    for blk01 in range(2):
                        bsl = slice(blk01 * 128, blk01 * 128 + 128)
                        if ch < NCH - 1:
                            tpk = ps_small([128, 128], F16)
                            nc.tensor.transpose(tpk[:], kc[:, hl, bsl], i16[:])
                            kn = rpool.tile([128, 128], F16, tag="knat")
                            nc.scalar.copy(kn[:], tpk[:])
                            knat.append(kn)
                        vv = rpool.tile([128, 128], F16, tag="vv")
                        nc.vector.tensor_scalar(
                            vv[:], vcn[blk01][:, hl * 128:(hl + 1) * 128],
                            vf[:, ch, blk01, hl:hl + 1], None, ALU.mult)
                        vvt.append(vv)
                    # AT (masked): rows cj, cols ci
                    at0ps = ps_small([128, 256])
                    nc.tensor.matmul(at0ps[:], kc[:, hl, 0:128], qc[:, hl, :],
                                     start=True, stop=True)
                    at0 = rpool.tile([128, CS], F16, tag="at0")
                    nc.vector.scalar_tensor_tensor(
                        at0[:, 0:128], at0ps[:, 0:128], 1.0, Um32,
                        op0=ALU.mult, op1=ALU.mult)
                    nc.scalar.copy(at0[:, 128:256], at0ps[:, 128:256])
                    at1ps = ps_small([128, 128])
                    nc.tensor.matmul(at1ps[:], kc[:, hl, 128:256],
                                     qc[:, hl, 128:256], start=True, stop=True)
                    at1 = rpool.tile([128, 128], F16, tag="at1s")
                    nc.vector.scalar_tensor_tensor(
                        at1[:], at1ps[:], 1.0, Um32, op0=ALU.mult, op1=ALU.mult)
                    # o = intra + inter (one PSUM group per ci half)
                    for ci in range(2):
                        csl = slice(ci * 128, ci * 128 + 128)
                        mms = [(at0[:, csl], vvt[0][:])]
                        if ci == 1:
                            mms.append((at1[:], vvt[1][:]))
                        if ch > 0:
                            mms.append((qc[:, hl, csl], S_prev[hl][:]))
                        ops = ps_small([128, HD])
                        for i, (lh, rh) in enumerate(mms):
                            nc.tensor.matmul(ops[:], lh, rh, start=(i == 0),
                                             stop=(i == len(mms) - 1))
                        nc.scalar.mul(o_st[:, ci * HPC + hl, :], ops[:],
                                      rf[:, ch, ci, hl:hl + 1])
                    # state update: S_cur = (S_prev + contrib) * eS
                    if ch < NCH - 1:
                        sps = ps_small([128, HD])
                        nc.tensor.matmul(sps[:], knat[0][:], vvt[0][:],
                                         start=True, stop=False)
                        nc.tensor.matmul(sps[:], knat[1][:], vvt[1][:],
                                         start=False, stop=True)
                        S_cur = rpool.tile([128, HD], F16, tag=f"S{hl}")
                        if ch > 0:
                            stmp = rpool.tile([128, HD], F32, tag="stmp")
                            nc.vector.tensor_add(stmp[:], S_prev[hl][:], sps[:])
                            nc.vector.tensor_scalar(
                                S_cur[:], stmp[:], eS[:, ch, hl:hl + 1], None,
                                ALU.mult)
                        else:
                            nc.vector.tensor_scalar(
                                S_cur[:], sps[:], eS[:, ch, hl:hl + 1], None,
                                ALU.mult)
                        S_prev[hl] = S_cur
                # o-norm over head dim (free)
                osq = rpool.tile([128, 2 * HPC, HD], F32, tag="osq", bufs=1)
                nc.scalar.activation(osq[:], o_st[:], ACTF.Square)
                ssum = rpool.tile([128, 2 * HPC], F32, tag="ossum")
                nc.vector.tensor_reduce(ssum[:], osq[:], AX.X, ALU.add)
                nc.vector.tensor_scalar(ssum[:], ssum[:], 1.0 / HD, EPS,
                                        ALU.mult, ALU.add)
                nc.vector.reciprocal(ssum[:], ssum[:])
                nc.scalar.activation(ssum[:], ssum[:], ACTF.Sqrt)
                o_n = rpool.tile([128, 2 * HPC, HD], F16, tag="o_n", bufs=2)
                nc.vector.tensor_tensor(
                    o_n[:], o_st[:],
                    ssum[:].unsqueeze(2).to_broadcast([128, 2 * HPC, HD]),
                    ALU.mult)
                # transpose + gate into go_st
                go_st = rpool.tile([128, HPC, CS], F16, tag="go_st")
                for hl in range(HPC):
                    for blk01 in range(2):
                        trp = ps_small([128, 128], F16)
                        nc.tensor.transpose(
                            trp[:], o_n[:][:, blk01 * HPC + hl, :], i16[:])
                        bsl = slice(blk01 * 128, blk01 * 128 + 128)
                        nc.vector.tensor_mul(
                            go_st[:, hl, bsl], trp[:], sg[blk01][:, hl, :])
                # out-proj for this chunk's two token tiles
                for m01 in range(2):
                    msl = slice(m01 * 128, m01 * 128 + 128)
                    for n in range(DIM // 512):
                        ps = ps_big()
                        nsl = slice(n * 512, (n + 1) * 512)
                        for k in range(HPC):
                            nc.tensor.matmul(ps[:], go_st[:, k, msl],
                                             wo_sb[:, k, nsl],
                                             start=(k == 0), stop=(k == HPC - 1))
                        oo = epool.tile([128, 512], F16, tag="oo", bufs=4)
                        if n % 2 == 0:
                            nc.vector.tensor_copy(oo[:], ps[:])
                            _dma_gpsimd(
                                out[ch * CS + m01 * 128:
                                    ch * CS + m01 * 128 + 128, nsl], oo[:])
                        else:
                            nc.scalar.copy(oo[:], ps[:])
                            _dma_scalar(
                                out[ch * CS + m01 * 128:
                                    ch * CS + m01 * 128 + 128, nsl], oo[:])

    nc.compile()
    return nc


def _prep_inputs(x, c, Wq, Wk, Wv, Wg, Wgt, Wo):
    """Build the 8 per-core input maps (host-side sharding / layout)."""
    consts = np.ascontiguousarray(_consts_np())
    c16 = np.eye(128, dtype=np.float16)
    in_maps = []
    xTs = [np.ascontiguousarray(x[b].T) for b in range(B)]
    cTs = [np.ascontiguousarray(c[b].T) for b in range(B)]
    for core in range(NCORE):
        b, g = core // 4, core % 4
        cols = slice(g * PCOLS, (g + 1) * PCOLS)
        heads = slice(g * HPC, (g + 1) * HPC)
        in_maps.append({
            "xT": xTs[b],
            "cT": cTs[b],
            "wq": np.ascontiguousarray(Wq[:, cols]),
            "wk": np.ascontiguousarray(Wk[:, cols]),
            "wv": np.ascontiguousarray(Wv[:, cols]),
            "wg": np.ascontiguousarray(Wg[:, cols]),
            "wgt": np.ascontiguousarray(Wgt[:, heads]),
            "wo": np.ascontiguousarray(Wo[cols, :]).astype(np.float16),
            "consts": consts,
            "c16": c16,
        })
    return in_maps


def kernel(x, c, Wq, Wk, Wv, Wg, Wgt, Wo, _want_results=False):
    key = "nc_dbg" if DEBUG else "nc"
    if key not in _cache:
        _cache[key] = build(debug=DEBUG)
    nc = _cache[key]
    in_maps = _prep_inputs(np.asarray(x, np.float32), np.asarray(c, np.float32),
                           np.asarray(Wq, np.float32), np.asarray(Wk, np.float32),
                           np.asarray(Wv, np.float32), np.asarray(Wg, np.float32),
                           np.asarray(Wgt, np.float32), np.asarray(Wo, np.float32))
    res = bass_utils.run_bass_kernel_spmd(
        nc, in_maps, core_ids=list(range(NCORE)), trace=TRACE)
    out = np.zeros((B, T, DIM), np.float32)
    for core in range(NCORE):
        out[core // 4] += res.results[core]["out"].astype(np.float32)
    if _want_results:
        return out, res
    return out



# revision 25
# speedup vs baseline: 1.3862x; 1.3862x over previous
"""GateRetention Trainium2 kernel (Bass/Tile), 8-core tensor-parallel, v2.

Sharding: core grid (batch b = core//4, head-group g = core%4); each core owns
4 heads (512 cols of the q/k/v/g projections, 512 rows of Wo) of one batch.
RMS-norm statistics AND the gate-logit projection (K-sharded over the 4 TP
cores) ride one AllReduce per token-half; out-proj partials summed on host.

v2 changes vs v1: fp16 projections in a single pass over x (x loaded once);
gt K-sharded (64 instead of 256 tensor instructions); AllReduce split into two
token-halves, each hidden under later compute; P3 folds the decay factor into
the A^T / k_nat evacuations (no separate vv tiles), computes the o-norm via
Square+accum_out straight from PSUM, folds rowfac+norm into one per-column
evac factor, and software-pipelines the out-proj one chunk behind retention.

kernel(**inputs) takes the FULL inputs from reference.setup_inputs() and
returns the FULL [B, T, DIM] fp32 output.
"""
import os
import sys

sys.path.insert(0, "/opt/trn_rl_repo")

import numpy as np

import concourse.bass as bass
import concourse.bacc as bacc
import concourse.tile as tile
import concourse.mybir as mybir
from concourse import bass_utils

F32 = mybir.dt.float32
F32R = mybir.dt.float32r
F16 = mybir.dt.float16
AX = mybir.AxisListType
ALU = mybir.AluOpType
ACTF = mybir.ActivationFunctionType

B, T, DIM = 2, 4096, 2048
H, HD = 16, 128
CS = 256
NCH = T // CS              # 16 chunks
EPS = 1e-5
GLN = 16.0
SCALE = HD ** -0.5
NCORE = 8
HPC = 4                    # heads per core
PCOLS = HPC * HD           # 512 cols per core
NBLK = T // 128            # 32 token blocks of 128
TSEG = T // 2              # 2048 tokens per AllReduce segment
NTS = TSEG // 512          # 4 token n-tiles per segment
SROWS = 3 + H              # sumsq q/k/v + 16 gt logit rows
VSH = 2.0 ** -4            # fp16 range shift on decayed tensors

DEBUG = bool(int(os.environ.get("GR_DEBUG", "0")))
TRACE = bool(int(os.environ.get("GR_TRACE", "0")))

_cache = {}


def _consts_np():
    """fp32 consts [128, 512]: identity | Lm | Om | Lc."""
    ident = np.eye(128, dtype=np.float32)
    jj, ii = np.meshgrid(np.arange(128), np.arange(128), indexing="ij")
    Lm = np.where(jj <= ii, -1.0 / GLN, 0.0).astype(np.float32)
    Om = np.full((128, 128), -1.0 / GLN, np.float32)
    # Lc: b' for even blocks = +1/GLN * sum_{j>i} sp_j (decay i -> block end)
    Lc = np.where(jj > ii, 1.0 / GLN, 0.0).astype(np.float32)
    return np.concatenate([ident, Lm, Om, Lc], axis=1)


def _consts16_np():
    """fp16 consts [128, 264]: identity | Um (tril ones) | ones."""
    ident = np.eye(128, dtype=np.float16)
    jj, ii = np.meshgrid(np.arange(128), np.arange(128), indexing="ij")
    Um = np.where(jj <= ii, 1.0, 0.0).astype(np.float16)
    ones = np.ones((128, 8), np.float16)
    return np.concatenate([ident, Um, ones], axis=1)


def build(debug=False):
    nc = bacc.Bacc("TRN2", target_bir_lowering=False, debug=False,
                   enable_asserts=False, num_devices=NCORE)

    # ---------------- I/O ----------------
    xT = nc.dram_tensor("xT", [DIM, T], F16, kind="ExternalInput").ap()
    xgt = nc.dram_tensor("xgt", [PCOLS, T], F16, kind="ExternalInput").ap()
    cgt = nc.dram_tensor("cgt", [PCOLS, T], F16, kind="ExternalInput").ap()
    wq = nc.dram_tensor("wq", [DIM, PCOLS], F16, kind="ExternalInput").ap()
    wk = nc.dram_tensor("wk", [DIM, PCOLS], F16, kind="ExternalInput").ap()
    wv = nc.dram_tensor("wv", [DIM, PCOLS], F16, kind="ExternalInput").ap()
    wg = nc.dram_tensor("wg", [DIM, PCOLS], F16, kind="ExternalInput").ap()
    wgt = nc.dram_tensor("wgt", [PCOLS, H], F16, kind="ExternalInput").ap()
    wo = nc.dram_tensor("wo", [PCOLS, DIM], F16, kind="ExternalInput").ap()
    consts = nc.dram_tensor("consts", [128, 512], F32R,
                            kind="ExternalInput").ap()
    c16 = nc.dram_tensor("c16", [128, 264], F16, kind="ExternalInput").ap()
    selT = nc.dram_tensor("selT", [SROWS, 8], F32, kind="ExternalInput").ap()
    out = nc.dram_tensor("out", [T, DIM], F16, kind="ExternalOutput").ap()

    def dbg(name, shape, dtype=F32):
        return nc.dram_tensor(name, shape, dtype, kind="ExternalOutput").ap()

    with tile.TileContext(nc) as tc:
        with (
            tc.tile_pool(name="const", bufs=1) as cpool,
            tc.tile_pool(name="wts", bufs=1) as wpool,
            tc.tile_pool(name="xstream", bufs=2) as xpool,
            tc.tile_pool(name="gstream", bufs=2) as gxpool,
            tc.tile_pool(name="evac", bufs=2) as epool,
            tc.tile_pool(name="persist", bufs=1) as ppool,
            tc.tile_pool(name="small", bufs=2) as spool,
            tc.tile_pool(name="ret", bufs=2) as rpool,
            tc.tile_pool(name="ps", bufs=1, space="PSUM") as psp,
        ):
            # PSUM buffers are whole 2KB banks: at most 8 concurrent.
            # big 2 + wide 2 + ops 2 + ops2 1 + tp 1 = 8.
            def ps_big():
                return psp.tile([128, 512], F32, tag="big", bufs=2,
                                name="psbig")

            def ps_wide(shape):
                return psp.tile(shape, F32, tag="wide", bufs=2, name="pswide")

            def ps_o(shape=None):
                return psp.tile(shape or [128, HD], F32, tag="ops", bufs=2,
                                name="pso")

            def ps_tp():
                return psp.tile([128, 128], F16, tag="tp", bufs=2, name="pstp")

            # ---------------- constants ----------------
            cst = cpool.tile([128, 512], F32R, tag="consts")
            nc.sync.dma_start(cst[:], consts)
            ident32 = cst[:, 0:128].bitcast(F32)
            Lm = cst[:, 128:256]
            Om = cst[:, 256:384]
            Lc = cst[:, 384:512]
            cst16 = cpool.tile([128, 264], F16, tag="c16")
            nc.sync.dma_start(cst16[:], c16)
            i16 = cst16[:, 0:128]
            Um16 = cst16[:, 128:256]
            ones16 = cst16[:, 256:257]
            selt = cpool.tile([SROWS, 8], F32, tag="selt")
            nc.sync.dma_start(selt[:], selT)

            # ---------------- DRAM scratch (fp16) ----------------
            if debug:
                qT_s = dbg("dbg_qT", [PCOLS, T], F16)
                kT_s = dbg("dbg_kT", [PCOLS, T], F16)
                vN_s = dbg("dbg_vN", [T, PCOLS], F16)
                gT_s = dbg("dbg_gT", [PCOLS, T], F16)
            else:
                qT_s = nc.dram_tensor("qT_s", [PCOLS, T], F16,
                                      kind="Internal").ap()
                kT_s = nc.dram_tensor("kT_s", [PCOLS, T], F16,
                                      kind="Internal").ap()
                vN_s = nc.dram_tensor("vN_s", [T, PCOLS], F16,
                                      kind="Internal").ap()
                gT_s = nc.dram_tensor("gT_s", [PCOLS, T], F16,
                                      kind="Internal").ap()
            ss_in = [nc.dram_tensor(f"ss_in{s}", [SROWS, TSEG], F32,
                                    kind="Internal").ap() for s in range(2)]
            ss_out = [nc.dram_tensor(f"ss_out{s}", [SROWS, TSEG], F32,
                                     kind="Internal").ap() for s in range(2)]

            # ---------------- weights ----------------
            def load_w(wdram, tag):
                wt = wpool.tile([128, 16, PCOLS], F16, tag=tag)
                nc.sync.dma_start(
                    wt[:], wdram.rearrange("(kt p) m -> p kt m", p=128))
                return wt

            wq_sb = load_w(wq, "wq")
            wk_sb = load_w(wk, "wk")
            wg_sb = load_w(wg, "wg")
            wv_sb = load_w(wv, "wv")
            wgt_sb = wpool.tile([128, 4, H], F16, tag="wgt")
            nc.sync.dma_start(wgt_sb[:],
                              wgt.rearrange("(kt p) m -> p kt m", p=128))

            # persistent P2 state
            vss = ppool.tile([128, NBLK], F32, tag="vss")
            gtd = ppool.tile([128, NCH, 2, HPC], F32R, tag="gtd")
            rf = ppool.tile([128, NCH, 2, HPC], F32, tag="rf")
            vf = ppool.tile([128, NCH, 2, HPC], F32, tag="vf")
            eS = ppool.tile([128, NCH - 1, HPC], F32, tag="eS")
            eSb7 = ppool.tile([128, HPC], F32, tag="eSb7")

            # =========================================================
            # P1: one pass over x -> q,k,g (T-layout), v (natural), gt
            # =========================================================
            def p1_ntile(nt, seg):
                tok = slice(nt * 512, (nt + 1) * 512)
                ltok = slice((nt - seg * NTS) * 512, (nt - seg * NTS + 1) * 512)
                xt = xpool.tile([128, 2, 8, 512], F16, tag="xt")
                nc.sync.dma_start(
                    xt[:], xT[:, tok].rearrange("(h k p) m -> p h k m",
                                                p=128, k=8))
                xg = gxpool.tile([128, 4, 512], F16, tag="xg")
                nc.sync.dma_start(
                    xg[:], xgt[:, tok].rearrange("(k p) m -> p k m", p=128))
                cg = gxpool.tile([128, 4, 512], F16, tag="cg")
                nc.sync.dma_start(
                    cg[:], cgt[:, tok].rearrange("(k p) m -> p k m", p=128))

                # q, k: T-layout + sumsq over channels
                for pi, (w_sb, sdram, row) in enumerate(
                        ((wq_sb, qT_s, 0), (wk_sb, kT_s, 1))):
                    ssps = None
                    for m in range(4):
                        msl = slice(m * 128, (m + 1) * 128)
                        ps = ps_big()
                        for kk in range(16):
                            nc.tensor.matmul(ps[:], w_sb[:, kk, msl],
                                             xt[:, kk // 8, kk % 8, :],
                                             start=(kk == 0), stop=(kk == 15))
                        ev = epool.tile([128, 512], F16, tag="ev", bufs=4)
                        if m % 2 == 0:
                            nc.vector.tensor_copy(ev[:], ps[:])
                            nc.sync.dma_start(
                                sdram[m * 128:(m + 1) * 128, tok], ev[:])
                        else:
                            nc.scalar.copy(ev[:], ps[:])
                            nc.scalar.dma_start(
                                sdram[m * 128:(m + 1) * 128, tok], ev[:])
                        sqt = epool.tile([128, 512], F16, tag="sq", bufs=3)
                        nc.scalar.activation(sqt[:], ps[:], ACTF.Square)
                        if m == 0:
                            ssps = ps_wide([1, 512])
                        nc.tensor.matmul(ssps[:1, :], ones16, sqt[:],
                                         start=(m == 0), stop=(m == 3))
                    ssev = spool.tile([1, 512], F32, tag="ssev", bufs=2)
                    nc.vector.tensor_copy(ssev[:], ssps[:1, :])
                    nc.sync.dma_start(ss_in[seg][row:row + 1, ltok], ssev[:])

                # g: T-layout, silu fused into evac
                for m in range(4):
                    msl = slice(m * 128, (m + 1) * 128)
                    ps = ps_big()
                    for kk in range(16):
                        nc.tensor.matmul(ps[:], wg_sb[:, kk, msl],
                                         xt[:, kk // 8, kk % 8, :],
                                         start=(kk == 0), stop=(kk == 15))
                    ev = epool.tile([128, 512], F16, tag="ev", bufs=4)
                    nc.scalar.activation(ev[:], ps[:], ACTF.Silu)
                    nc.scalar.dma_start(gT_s[m * 128:(m + 1) * 128, tok],
                                        ev[:])

                # v: natural layout + sumsq via accum
                for mt in range(4):
                    msl = slice(mt * 128, (mt + 1) * 128)
                    ps = ps_big()
                    for kk in range(16):
                        nc.tensor.matmul(ps[:], xt[:, kk // 8, kk % 8, msl],
                                         wv_sb[:, kk, :],
                                         start=(kk == 0), stop=(kk == 15))
                    ev = epool.tile([128, 512], F16, tag="ev", bufs=4)
                    sqv = epool.tile([128, 512], F16, tag="sq", bufs=3)
                    nc.scalar.activation(
                        sqv[:], ps[:], ACTF.Square,
                        accum_out=vss[:, nt * 4 + mt:nt * 4 + mt + 1])
                    if mt % 2 == 0:
                        nc.vector.tensor_copy(ev[:], ps[:])
                        nc.sync.dma_start(
                            vN_s[nt * 512 + mt * 128:nt * 512 + (mt + 1) * 128,
                                 :], ev[:])
                    else:
                        nc.scalar.copy(ev[:], ps[:])
                        nc.scalar.dma_start(
                            vN_s[nt * 512 + mt * 128:nt * 512 + (mt + 1) * 128,
                                 :], ev[:])

                # gt logits, K-sharded: this core's 512 rows of x+c
                gtps = ps_wide([H, 512])
                for kk in range(4):
                    nc.tensor.matmul(gtps[:H, :], wgt_sb[:, kk, :],
                                     xg[:, kk, :], start=(kk == 0), stop=False)
                for kk in range(4):
                    nc.tensor.matmul(gtps[:H, :], wgt_sb[:, kk, :],
                                     cg[:, kk, :], start=False,
                                     stop=(kk == 3))
                gev = spool.tile([H, 512], F32, tag="gev", bufs=2)
                nc.vector.tensor_copy(gev[:], gtps[:H, :])
                nc.sync.dma_start(ss_in[seg][3:3 + H, ltok], gev[:])

            def p1_seg_wrap(seg):
                # v sumsq: transpose this segment's 16 block-columns to a row
                vssT = ps_o([128, 128])
                nc.tensor.matmul(
                    vssT[:16, :],
                    vss[:, seg * 16:(seg + 1) * 16], ident32,
                    is_transpose=True)
                vssev = spool.tile([16, 128], F32, tag="vssev", bufs=2)
                nc.vector.tensor_copy(vssev[:], vssT[:16, :])
                nc.sync.dma_start(
                    ss_in[seg][2:3, :].rearrange("a (b c) -> (a b) c", c=128),
                    vssev[:])

            # =========================================================
            # P2 (per segment): scales + gate decays
            # =========================================================
            def p2_seg(seg):
                chs = slice(seg * 8, seg * 8 + 8)
                srt = spool.tile([SROWS, TSEG], F32, tag="srt", bufs=1)
                nc.sync.dma_start(srt[:], ss_out[seg])
                ssel = ppool.tile([128, 16, 8], F32, tag=f"ssel{seg}")
                for j in range(16):
                    tp = ps_o([128, 8])
                    nc.tensor.matmul(tp[:], srt[:, j * 128:(j + 1) * 128],
                                     selt[:], start=True, stop=True)
                    nc.vector.tensor_copy(ssel[:, j, :], tp[:])
                # rsn = (ms/DIM + EPS)^-0.5  (1/DIM folded into selT)
                rsn = spool.tile([128, 16, 3], F32, tag="rsn", bufs=2)
                nc.vector.tensor_scalar(rsn[:], ssel[:, :, 0:3], 1.0, EPS,
                                        ALU.mult, ALU.add)
                nc.scalar.activation(rsn[:], rsn[:], ACTF.Ln)
                nc.scalar.activation(rsn[:], rsn[:], ACTF.Exp, scale=-0.5)
                # rsq = rsn_q * SCALE / VSH ; skv = rsn_k * rsn_v * VSH
                # ([128, 8, 2] so even/odd blocks are plain slices)
                rsq = spool.tile([128, 8, 2], F32, tag="rsq", bufs=2)
                nc.vector.tensor_scalar(
                    rsq[:].rearrange("p a b -> p (a b)"), rsn[:, :, 0],
                    SCALE / VSH, None, ALU.mult)
                skv = spool.tile([128, 8, 2], F32, tag="skv", bufs=2)
                skv_f = skv[:].rearrange("p a b -> p (a b)")
                nc.vector.tensor_mul(skv_f, rsn[:, :, 1], rsn[:, :, 2])
                nc.vector.tensor_scalar(skv_f, skv_f, VSH, None, ALU.mult)
                # gtd = softplus(-z) = ln(1 + exp(-z)); selT folds the -1
                gt_view = gtd[:, chs].rearrange("p a b c -> p (a b) c")
                nc.scalar.activation(gt_view, ssel[:, :, 3:7], ACTF.Exp)
                nc.scalar.activation(gt_view, gt_view, ACTF.Ln, bias=1.0)

                # recentred decays, batched over the segment's 8 chunks.
                # rf = exp(b')*rsq ; vf = exp(-b')*skv per block; each small
                # PSUM is consumed before the next is filled (2 bufs).
                for b01, tri in ((0, Lc), (1, Lm)):
                    pp = ps_o([128, 8, HPC])
                    nc.tensor.matmul(pp[:], tri, gtd[:, chs, b01, :],
                                     start=True, stop=True)
                    ex = spool.tile([128, 8, HPC], F32, tag="p2e", bufs=4)
                    nc.scalar.activation(ex[:], pp[:], ACTF.Exp)
                    nc.vector.tensor_tensor(
                        rf[:, chs, b01, :], ex[:],
                        rsq[:, :, b01].unsqueeze(2).to_broadcast(
                            [128, 8, HPC]), ALU.mult)
                    ex2 = spool.tile([128, 8, HPC], F32, tag="p2e", bufs=4)
                    nc.scalar.activation(ex2[:], pp[:], ACTF.Exp, scale=-1.0)
                    nc.vector.tensor_tensor(
                        vf[:, chs, b01, :], ex2[:],
                        skv[:, :, b01].unsqueeze(2).to_broadcast(
                            [128, 8, HPC]), ALU.mult)
                ptv = ps_o([128, 8, HPC])
                nc.tensor.matmul(ptv[:], Om, gtd[:, chs, 1, :],
                                 start=True, stop=False)
                nc.tensor.matmul(ptv[:, 0:7, :], Om,
                                 gtd[:, seg * 8 + 1:seg * 8 + 8, 0, :],
                                 start=False, stop=True, skip_group_check=True)
                if seg == 0:
                    # cols 0..6 complete; col 7 lacks block 16 (next segment)
                    nc.scalar.activation(eS[:, 0:8, :], ptv[:], ACTF.Exp)
                else:
                    nc.scalar.activation(eS[:, 8:15, :], ptv[:, 0:7, :],
                                         ACTF.Exp)
                    pb = ps_o([128, HPC])
                    nc.tensor.matmul(pb[:], Om, gtd[:, 8, 0, :],
                                     start=True, stop=True)
                    nc.scalar.activation(eSb7[:], pb[:], ACTF.Exp)

            # =========================================================
            # P3: retention + gating + out-proj, out-proj 1 chunk behind
            # =========================================================
            # reuse wq's SBUF (P1 done by the time the load lands)
            wo_sb = wpool.tile([128, HPC, DIM], F16, tag="wq")

            S_prev = [None] * HPC

            def p3_retention(ch):
                tok = slice(ch * CS, (ch + 1) * CS)
                qc = rpool.tile([128, HPC, CS], F16, tag="qc")
                kc = rpool.tile([128, HPC, CS], F16, tag="kc")
                for t_, s_ in ((qc, qT_s), (kc, kT_s)):
                    nc.sync.dma_start(
                        t_[:], s_[:, tok].rearrange("(h p) m -> p h m", p=128))
                vcn, sg = [], []
                for b01 in range(2):
                    bt = slice(ch * CS + b01 * 128, ch * CS + b01 * 128 + 128)
                    vt = rpool.tile([128, PCOLS], F16, tag="vcn", bufs=4)
                    nc.sync.dma_start(vt[:], vN_s[bt, :])
                    vcn.append(vt)
                    gt_ = rpool.tile([128, HPC, 128], F16, tag="gch", bufs=4)
                    nc.sync.dma_start(
                        gt_[:], gT_s[:, bt].rearrange("(h p) m -> p h m",
                                                      p=128))
                    sg.append(gt_)
                if ch == 8:
                    # deferred cross-segment state decay (block 16 part)
                    for hl in range(HPC):
                        Sn = rpool.tile([128, HD], F16, tag=f"S{hl}", bufs=3)
                        nc.vector.tensor_scalar(
                            Sn[:], S_prev[hl][:], eSb7[:, hl:hl + 1], None,
                            ALU.mult)
                        S_prev[hl] = Sn
                S_old = list(S_prev)
                # phase A: k transposes (decayed), A^T (masked, decayed),
                # state update
                at0s, at1s = [], []
                for hl in range(HPC):
                    knat = []
                    if ch < NCH - 1:
                        for b01 in range(2):
                            bsl = slice(b01 * 128, b01 * 128 + 128)
                            tpk = ps_tp()
                            nc.tensor.transpose(tpk[:], kc[:, hl, bsl],
                                                i16[:])
                            kn = rpool.tile([128, 128], F16, tag="knat",
                                            bufs=4)
                            nc.scalar.mul(kn[:], tpk[:],
                                          vf[:, ch, b01, hl:hl + 1])
                            knat.append(kn)
                    atps = ps_wide([128, 384])
                    nc.tensor.matmul(atps[:, 0:256], kc[:, hl, 0:128],
                                     qc[:, hl, :], start=True, stop=True)
                    nc.tensor.matmul(atps[:, 256:384], kc[:, hl, 128:256],
                                     qc[:, hl, 128:256], start=True, stop=True,
                                     skip_group_check=True)
                    at0 = rpool.tile([128, CS], F16, tag="at0", bufs=5)
                    nc.vector.tensor_scalar(at0[:], atps[:, 0:256],
                                            vf[:, ch, 0, hl:hl + 1], None,
                                            ALU.mult)
                    nc.vector.tensor_mul(at0[:, 0:128], at0[:, 0:128], Um16)
                    at1 = rpool.tile([128, 128], F16, tag="at1", bufs=5)
                    nc.vector.tensor_scalar(at1[:], atps[:, 256:384],
                                            vf[:, ch, 1, hl:hl + 1], None,
                                            ALU.mult)
                    nc.vector.tensor_mul(at1[:], at1[:], Um16)
                    at0s.append(at0)
                    at1s.append(at1)
                    if ch < NCH - 1:
                        sps = ps_o()
                        nc.tensor.matmul(sps[:], knat[0][:],
                                         vcn[0][:, hl * 128:(hl + 1) * 128],
                                         start=True, stop=False)
                        nc.tensor.matmul(sps[:], knat[1][:],
                                         vcn[1][:, hl * 128:(hl + 1) * 128],
                                         start=False, stop=True)
                        S_cur = rpool.tile([128, HD], F16, tag=f"S{hl}",
                                           bufs=3)
                        if ch > 0:
                            stmp = rpool.tile([128, HD], F32, tag="stmp",
                                              bufs=2)
                            nc.vector.tensor_add(stmp[:], S_prev[hl][:],
                                                 sps[:])
                            nc.vector.tensor_scalar(
                                S_cur[:], stmp[:], eS[:, ch, hl:hl + 1], None,
                                ALU.mult)
                        else:
                            nc.vector.tensor_scalar(
                                S_cur[:], sps[:], eS[:, ch, hl:hl + 1], None,
                                ALU.mult)
                        S_prev[hl] = S_cur
                # phase B/C: o per ci-half; raw o evacuated fp16, then the
                # rowfac+rmsnorm factor F applied as one per-column multiply
                o_r = rpool.tile([128, 2 * HPC, HD], F16, tag="o_r")
                o_n = rpool.tile([128, 2 * HPC, HD], F16, tag="o_n")
                msq = rpool.tile([128, 2, HPC], F32, tag="msq")
                for ci in range(2):
                    csl = slice(ci * 128, ci * 128 + 128)
                    for hl in range(HPC):
                        mms = [(at0s[hl][:, csl],
                                vcn[0][:, hl * 128:(hl + 1) * 128])]
                        if ci == 1:
                            mms.append((at1s[hl][:],
                                        vcn[1][:, hl * 128:(hl + 1) * 128]))
                        if ch > 0:
                            mms.append((qc[:, hl, csl], S_old[hl][:]))
                        ops = ps_o()
                        for i, (lh, rh) in enumerate(mms):
                            nc.tensor.matmul(ops[:], lh, rh, start=(i == 0),
                                             stop=(i == len(mms) - 1))
                        osl = o_r[:, ci * HPC + hl, :]
                        nc.scalar.copy(osl, ops[:])
                        sqs = rpool.tile([128, HD], F32, tag="sqs", bufs=2)
                        nc.scalar.activation(sqs[:], osl, ACTF.Square,
                                             accum_out=msq[:, ci, hl:hl + 1])
                    # F = rf * (rf^2 * msq / HD + EPS)^-0.5 for 4 heads
                    Ft = rpool.tile([128, HPC], F32, tag="Ft", bufs=4)
                    nc.vector.tensor_mul(Ft[:], rf[:, ch, ci, :],
                                         rf[:, ch, ci, :])
                    nc.vector.tensor_mul(Ft[:], Ft[:], msq[:, ci, :])
                    nc.vector.tensor_scalar(Ft[:], Ft[:], 1.0 / HD, EPS,
                                            ALU.mult, ALU.add)
                    nc.vector.reciprocal(Ft[:], Ft[:])
                    nc.scalar.activation(Ft[:], Ft[:], ACTF.Sqrt)
                    nc.vector.tensor_mul(Ft[:], Ft[:], rf[:, ch, ci, :])
                    for hl in range(HPC):
                        nc.vector.tensor_scalar(
                            o_n[:, ci * HPC + hl, :], o_r[:, ci * HPC + hl, :],
                            Ft[:, hl:hl + 1], None, ALU.mult)
                return o_n, sg

            def p3_output(ch, o_n, sg):
                go_st = rpool.tile([128, HPC, CS], F16, tag="go_st")
                for hl in range(HPC):
                    for b01 in range(2):
                        trp = ps_tp()
                        nc.tensor.transpose(
                            trp[:], o_n[:][:, b01 * HPC + hl, :], i16[:])
                        bsl = slice(b01 * 128, b01 * 128 + 128)
                        nc.vector.tensor_mul(
                            go_st[:, hl, bsl], trp[:], sg[b01][:, hl, :])
                for m01 in range(2):
                    msl = slice(m01 * 128, m01 * 128 + 128)
                    for n in range(DIM // 512):
                        ps = ps_big()
                        nsl = slice(n * 512, (n + 1) * 512)
                        for k in range(HPC):
                            nc.tensor.matmul(ps[:], go_st[:, k, msl],
                                             wo_sb[:, k, nsl],
                                             start=(k == 0),
                                             stop=(k == HPC - 1))
                        oo = epool.tile([128, 512], F16, tag="oo", bufs=4)
                        if n % 2 == 0:
                            nc.vector.tensor_copy(oo[:], ps[:])
                            nc.sync.dma_start(
                                out[ch * CS + m01 * 128:
                                    ch * CS + m01 * 128 + 128, nsl], oo[:])
                        else:
                            nc.scalar.copy(oo[:], ps[:])
                            nc.scalar.dma_start(
                                out[ch * CS + m01 * 128:
                                    ch * CS + m01 * 128 + 128, nsl], oo[:])

            # ------------------ emission order ------------------
            for nt in range(NTS):
                p1_ntile(nt, 0)
            p1_seg_wrap(0)
            nc.gpsimd.collective_compute(
                "AllReduce", ALU.add,
                replica_groups=[[0, 1, 2, 3], [4, 5, 6, 7]],
                ins=[ss_in[0].opt()], outs=[ss_out[0].opt()],
            )
            for nt in range(NTS, 2 * NTS):
                p1_ntile(nt, 1)
            p1_seg_wrap(1)
            nc.gpsimd.collective_compute(
                "AllReduce", ALU.add,
                replica_groups=[[0, 1, 2, 3], [4, 5, 6, 7]],
                ins=[ss_in[1].opt()], outs=[ss_out[1].opt()],
            )
            nc.sync.dma_start(wo_sb[:],
                              wo.rearrange("(h p) m -> p h m", p=128))
            p2_seg(0)
            pend = None
            for ch in range(NCH):
                if ch == 8:
                    p2_seg(1)
                cur = p3_retention(ch)
                if pend is not None:
                    p3_output(pend[0], pend[1], pend[2])
                pend = (ch, cur[0], cur[1])
            p3_output(pend[0], pend[1], pend[2])

    nc.compile()
    return nc


def _prep_inputs(x, c, Wq, Wk, Wv, Wg, Wgt, Wo):
    """Build the 8 per-core input maps (host-side sharding / layout)."""
    consts = np.ascontiguousarray(_consts_np())
    c16 = np.ascontiguousarray(_consts16_np())
    in_maps = []
    xTs = [np.ascontiguousarray(x[b].T.astype(np.float16)) for b in range(B)]
    cTs = [np.ascontiguousarray(c[b].T.astype(np.float16)) for b in range(B)]
    Wgt16 = Wgt.astype(np.float16)
    for core in range(NCORE):
        b, g = core // 4, core % 4
        cols = slice(g * PCOLS, (g + 1) * PCOLS)
        sel = np.zeros((SROWS, 8), np.float32)
        for j in range(3):
            sel[j, j] = 1.0 / DIM
        for jj in range(HPC):
            sel[3 + 4 * g + jj, 3 + jj] = -1.0
        in_maps.append({
            "xT": xTs[b],
            "xgt": np.ascontiguousarray(xTs[b][cols.start:cols.stop, :]),
            "cgt": np.ascontiguousarray(cTs[b][cols.start:cols.stop, :]),
            "wq": np.ascontiguousarray(Wq[:, cols]).astype(np.float16),
            "wk": np.ascontiguousarray(Wk[:, cols]).astype(np.float16),
            "wv": np.ascontiguousarray(Wv[:, cols]).astype(np.float16),
            "wg": np.ascontiguousarray(Wg[:, cols]).astype(np.float16),
            "wgt": np.ascontiguousarray(Wgt16[cols, :]),
            "wo": np.ascontiguousarray(Wo[cols, :]).astype(np.float16),
            "consts": consts,
            "c16": c16,
            "selT": sel,
        })
    return in_maps


def kernel(x, c, Wq, Wk, Wv, Wg, Wgt, Wo, _want_results=False):
    key = "nc_dbg" if DEBUG else "nc"
    if key not in _cache:
        _cache[key] = build(debug=DEBUG)
    nc = _cache[key]
    in_maps = _prep_inputs(np.asarray(x, np.float32), np.asarray(c, np.float32),
                           np.asarray(Wq, np.float32), np.asarray(Wk, np.float32),
                           np.asarray(Wv, np.float32), np.asarray(Wg, np.float32),
                           np.asarray(Wgt, np.float32), np.asarray(Wo, np.float32))
    res = bass_utils.run_bass_kernel_spmd(
        nc, in_maps, core_ids=list(range(NCORE)), trace=TRACE)
    out = np.zeros((B, T, DIM), np.float32)
    for core in range(NCORE):
        out[core // 4] += res.results[core]["out"].astype(np.float32)
    if _want_results:
        return out, res
    return out
